# revision 6
# baseline (speedup 1.0000x reference)
"""Inverted-dropout kernel for Trainium2, distributed over 8 NeuronCores.

Computes out = where(mask, x * 2.0, 0) for x:(64,2048,4,7,7) f32 and
mask:(64,2048,4,7,7) bool.  Pure elementwise: shard along batch (8 per core),
each core streams its shard HBM->SBUF, applies one fused DVE op
(x * 2.0) * mask  (mask read directly as uint8), and streams back.
"""

import numpy as np

import concourse.bacc as bacc
import concourse.mybir as mybir
from concourse.tile import TileContext
from concourse import bass_utils

# Full problem shape (hardcoded per harness contract).
B, C, FM, H, W = 64, 2048, 4, 7, 7
N_CORES = 8
B_PER_CORE = B // N_CORES                       # 8
ELEMS_PER_CORE = B_PER_CORE * C * FM * H * W    # 3,211,264 = 7 * 128 * 3584

P = 128          # SBUF partitions
F = 3584         # free-dim elems per tile  (128*3584*4B = 1.75 MiB per x DMA)
NT = ELEMS_PER_CORE // (P * F)                  # 7 tiles
assert NT * P * F == ELEMS_PER_CORE

SCALE = 2.0      # 1 / (1 - p_drop), p_drop = 0.5
BUFS = 4


def build_nc(nt=NT, f=F, bufs=BUFS):
    # Bacc (not bare Bass): Bacc.compile() legalizes sync waits down to the
    # TRN2 1-wait-per-instruction limit — walrus rejects the module otherwise.
    nc = bacc.Bacc()
    x = nc.declare_dram_parameter("x", [nt, P, f], mybir.dt.float32, isOutput=False)
    m = nc.declare_dram_parameter("mask", [nt, P, f], mybir.dt.uint8, isOutput=False)
    o = nc.declare_dram_parameter("out", [nt, P, f], mybir.dt.float32, isOutput=True)
    with TileContext(nc) as tc:
        with tc.tile_pool(name="sbuf", bufs=bufs) as pool:
            for i in range(nt):
                xt = pool.tile([P, f], mybir.dt.float32)
                nc.sync.dma_start(out=xt[:], in_=x[i])
                mt = pool.tile([P, f], mybir.dt.uint8)
                nc.sync.dma_start(out=mt[:], in_=m[i])
                ot = pool.tile([P, f], mybir.dt.float32)
                # mask arrives pre-scaled to {0, 2} (host folds the 1/(1-p)
                # dropout scale into the byte mask), so one TensorTensor op
                # computes the whole dropout.
                nc.vector.tensor_mul(out=ot[:], in0=xt[:], in1=mt[:])
                nc.sync.dma_start(out=o[i], in_=ot[:])
    nc.compile()
    return nc


_NC_CACHE = {}


def _get_nc():
    if "nc" not in _NC_CACHE:
        _NC_CACHE["nc"] = build_nc()
    return _NC_CACHE["nc"]


def _shard_inputs(x: np.ndarray, mask: np.ndarray):
    # {0,1} bool bytes -> {0,2} u8: folds the dropout scale into the mask.
    mask_u8 = np.ascontiguousarray(mask).view(np.uint8) << 1
    in_maps = []
    for c in range(N_CORES):
        xs = np.ascontiguousarray(x[c * B_PER_CORE:(c + 1) * B_PER_CORE])
        ms = np.ascontiguousarray(mask_u8[c * B_PER_CORE:(c + 1) * B_PER_CORE])
        in_maps.append({
            "x": xs.reshape(NT, P, F),
            "mask": ms.reshape(NT, P, F),
        })
    return in_maps


def kernel(x: np.ndarray, mask: np.ndarray, **_) -> np.ndarray:
    nc = _get_nc()
    in_maps = _shard_inputs(x, mask)
    res = bass_utils.run_bass_kernel_spmd(nc, in_maps, list(range(N_CORES)))
    shards = [
        res.results[c]["out"].reshape(B_PER_CORE, C, FM, H, W)
        for c in range(N_CORES)
    ]
    return np.concatenate(shards, axis=0)


# revision 10
# speedup vs baseline: 24.2148x; 24.2148x over previous
"""Inverted-dropout kernel for Trainium2, distributed over 8 NeuronCores.

Computes out = where(mask, x * 2.0, 0) for x:(64,2048,4,7,7) f32 and
mask:(64,2048,4,7,7) bool.  Pure elementwise: shard along batch (8 per core),
each core streams its shard HBM->SBUF, applies one fused DVE op
(x * 2.0) * mask  (mask read directly as uint8), and streams back.
"""

import numpy as np

import concourse.bacc as bacc
import concourse.mybir as mybir
from concourse.tile import TileContext
from concourse import bass_utils

# Full problem shape (hardcoded per harness contract).
B, C, FM, H, W = 64, 2048, 4, 7, 7
N_CORES = 8
B_PER_CORE = B // N_CORES                       # 8
ELEMS_PER_CORE = B_PER_CORE * C * FM * H * W    # 3,211,264 = 7 * 128 * 3584

P = 128          # SBUF partitions
F = 3584         # free-dim elems per tile  (128*3584*4B = 1.75 MiB per x DMA)
NT = ELEMS_PER_CORE // (P * F)                  # 7 tiles
assert NT * P * F == ELEMS_PER_CORE

SCALE = 2.0      # 1 / (1 - p_drop), p_drop = 0.5
BUFS = 6


def build_nc(nt=NT, f=F, bufs=BUFS, repeat=1):
    """Build the per-core SPMD module.

    - Bacc (not bare Bass): Bacc.compile() legalizes sync waits down to the
      TRN2 1-wait-per-instruction limit — walrus rejects the module otherwise.
    - Loads and stores alternate between the two HWDGE rings (SP via nc.sync,
      ACT via nc.scalar) per tile, balancing bytes across both rings
      (~14.5 MB each instead of 16/12.8) — measured ~5% faster than putting
      all loads on one ring.
    - repeat>1 unrolls the whole body R times inside one NEFF (idempotent
      rewrites of the same output), used only for launch-overhead-free timing
      via (T(R2)-T(R1))/(R2-R1).
    """
    nc = bacc.Bacc()
    x = nc.declare_dram_parameter("x", [nt, P, f], mybir.dt.float32, isOutput=False)
    m = nc.declare_dram_parameter("mask", [nt, P, f], mybir.dt.uint8, isOutput=False)
    o = nc.declare_dram_parameter("out", [nt, P, f], mybir.dt.float32, isOutput=True)
    with TileContext(nc) as tc:
        with tc.tile_pool(name="sbuf", bufs=bufs) as pool:
            for _ in range(repeat):
                for i in range(nt):
                    load_eng = nc.sync if i % 2 == 0 else nc.scalar
                    store_eng = nc.scalar if i % 2 == 0 else nc.sync
                    xt = pool.tile([P, f], mybir.dt.float32)
                    load_eng.dma_start(out=xt[:], in_=x[i])
                    mt = pool.tile([P, f], mybir.dt.uint8)
                    load_eng.dma_start(out=mt[:], in_=m[i])
                    ot = pool.tile([P, f], mybir.dt.float32)
                    # mask arrives pre-scaled to {0, 2} (host folds the
                    # 1/(1-p) dropout scale into the byte mask), so one
                    # TensorTensor op computes the whole dropout.
                    nc.vector.tensor_mul(out=ot[:], in0=xt[:], in1=mt[:])
                    store_eng.dma_start(out=o[i], in_=ot[:])
    nc.compile()
    return nc


_NC_CACHE = {}


def _get_nc():
    if "nc" not in _NC_CACHE:
        _NC_CACHE["nc"] = build_nc()
    return _NC_CACHE["nc"]


def _shard_inputs(x: np.ndarray, mask: np.ndarray):
    # {0,1} bool bytes -> {0,2} u8: folds the dropout scale into the mask.
    mask_u8 = np.ascontiguousarray(mask).view(np.uint8) << 1
    in_maps = []
    for c in range(N_CORES):
        xs = np.ascontiguousarray(x[c * B_PER_CORE:(c + 1) * B_PER_CORE])
        ms = np.ascontiguousarray(mask_u8[c * B_PER_CORE:(c + 1) * B_PER_CORE])
        in_maps.append({
            "x": xs.reshape(NT, P, F),
            "mask": ms.reshape(NT, P, F),
        })
    return in_maps


def kernel(x: np.ndarray, mask: np.ndarray, **_) -> np.ndarray:
    nc = _get_nc()
    in_maps = _shard_inputs(x, mask)
    res = bass_utils.run_bass_kernel_spmd(nc, in_maps, list(range(N_CORES)))
    shards = [
        res.results[c]["out"].reshape(B_PER_CORE, C, FM, H, W)
        for c in range(N_CORES)
    ]
    return np.concatenate(shards, axis=0)


# revision 12
# speedup vs baseline: 35.9733x; 1.4856x over previous
"""Inverted-dropout kernel for Trainium2, distributed over 8 NeuronCores.

Computes out = where(mask, x * 2.0, 0) for x:(64,2048,4,7,7) f32 and
mask:(64,2048,4,7,7) bool.  Pure elementwise: shard along batch (8 per core).
Each core streams its shard HBM->SBUF in [128, 3584] f32 tiles, applies one
fused DVE TensorTensor op (the host pre-folds the 1/(1-p)=2.0 dropout scale
into the byte mask, so the op is just x * mask2 with mask2 in {0,2} read
directly as uint8), and streams the result back.  Loads/stores alternate
between the two HWDGE rings (SP / ACT) to balance DMA bytes across rings.

Measured ~68-70 us per core steady-state (~28.9 MB HBM traffic per core at
~420 GB/s — at the practical HBM/fabric ceiling; TimelineSim predicts 84 us).
"""

import sys

import numpy as np

try:
    import concourse.bacc as bacc
except ImportError:  # grading env without the default sys.path site config
    for p in ("/root/.axon_site/_ro/trn_rl_repo", "/opt/trn_rl_repo"):
        if p not in sys.path:
            sys.path.append(p)
    import concourse.bacc as bacc

import concourse.mybir as mybir
from concourse.tile import TileContext

# Full problem shape (hardcoded per harness contract).
B, C, FM, H, W = 64, 2048, 4, 7, 7
N_CORES = 8
B_PER_CORE = B // N_CORES                       # 8
ELEMS_PER_CORE = B_PER_CORE * C * FM * H * W    # 3,211,264 = 7 * 128 * 3584

P = 128          # SBUF partitions
F = 3584         # free-dim elems per tile  (128*3584*4B = 1.75 MiB per x DMA)
NT = ELEMS_PER_CORE // (P * F)                  # 7 tiles
assert NT * P * F == ELEMS_PER_CORE

SCALE = 2.0      # 1 / (1 - p_drop), p_drop = 0.5
BUFS = 6


def build_nc(nt=NT, f=F, bufs=BUFS, repeat=1):
    """Build the per-core SPMD module.

    - Bacc (not bare Bass): Bacc.compile() legalizes sync waits down to the
      TRN2 1-wait-per-instruction limit — walrus rejects the module otherwise.
    - Loads and stores alternate between the two HWDGE rings (SP via nc.sync,
      ACT via nc.scalar) per tile, balancing bytes across both rings
      (~14.5 MB each instead of 16/12.8) — measured ~5% faster than putting
      all loads on one ring.
    - repeat>1 unrolls the whole body R times inside one NEFF (idempotent
      rewrites of the same output), used only for launch-overhead-free timing
      via (T(R2)-T(R1))/(R2-R1).
    """
    nc = bacc.Bacc()
    x = nc.declare_dram_parameter("x", [nt, P, f], mybir.dt.float32, isOutput=False)
    m = nc.declare_dram_parameter("mask", [nt, P, f], mybir.dt.uint8, isOutput=False)
    o = nc.declare_dram_parameter("out", [nt, P, f], mybir.dt.float32, isOutput=True)
    with TileContext(nc) as tc:
        with tc.tile_pool(name="sbuf", bufs=bufs) as pool:
            for _ in range(repeat):
                for i in range(nt):
                    load_eng = nc.sync if i % 2 == 0 else nc.scalar
                    store_eng = nc.scalar if i % 2 == 0 else nc.sync
                    xt = pool.tile([P, f], mybir.dt.float32)
                    load_eng.dma_start(out=xt[:], in_=x[i])
                    mt = pool.tile([P, f], mybir.dt.uint8)
                    load_eng.dma_start(out=mt[:], in_=m[i])
                    ot = pool.tile([P, f], mybir.dt.float32)
                    # mask arrives pre-scaled to {0, 2} (host folds the
                    # 1/(1-p) dropout scale into the byte mask), so one
                    # TensorTensor op computes the whole dropout.
                    nc.vector.tensor_mul(out=ot[:], in0=xt[:], in1=mt[:])
                    store_eng.dma_start(out=o[i], in_=ot[:])
    nc.compile()
    return nc


def _build_runner(nc, n_cores):
    """Compile the SPMD module into a reusable shard_map-jitted callable.

    Same machinery as bass2jax.run_bass_via_pjrt, but the jitted function is
    built once and cached so repeated kernel() calls skip XLA re-tracing.
    Output-buffer donation is dropped: this kernel writes every output
    element, so zero-initialized outputs are unnecessary.
    """
    import jax
    from jax.sharding import Mesh, PartitionSpec, NamedSharding
    from jax.experimental.shard_map import shard_map
    from concourse.bass2jax import (
        _bass_exec_p,
        install_neuronx_cc_hook,
        partition_id_tensor,
    )

    install_neuronx_cc_hook()
    partition_name = nc.partition_id_tensor.name if nc.partition_id_tensor else None

    in_names, out_names, out_avals = [], [], []
    for alloc in nc.m.functions[0].allocations:
        if not isinstance(alloc, mybir.MemoryLocationSet):
            continue
        name = alloc.memorylocations[0].name
        if alloc.kind == "ExternalInput":
            if name != partition_name:
                in_names.append(name)
        elif alloc.kind == "ExternalOutput":
            out_names.append(name)
            out_avals.append(
                jax.core.ShapedArray(
                    tuple(alloc.tensor_shape), mybir.dt.np(alloc.dtype)
                )
            )
    n_params = len(in_names)
    all_in_names = list(in_names) + list(out_names)
    if partition_name is not None:
        all_in_names.append(partition_name)

    def _body(*args):
        operands = list(args)
        if partition_name is not None:
            operands.append(partition_id_tensor())
        outs = _bass_exec_p.bind(
            *operands,
            out_avals=tuple(out_avals),
            in_names=tuple(all_in_names),
            out_names=tuple(out_names),
            lowering_input_output_aliases=(),
            sim_require_finite=True,
            sim_require_nnan=True,
            nc=nc,
        )
        return tuple(outs)

    devices = jax.devices()[:n_cores]
    assert len(devices) == n_cores, (
        f"need {n_cores} devices, have {len(jax.devices())}"
    )
    mesh = Mesh(np.asarray(devices), ("core",))
    in_specs = (PartitionSpec("core"),) * (n_params + len(out_names))
    out_specs = (PartitionSpec("core"),) * len(out_names)
    fn = jax.jit(
        shard_map(
            _body, mesh=mesh, in_specs=in_specs, out_specs=out_specs,
            check_rep=False,
        ),
        keep_unused=True,
    )
    sharding = NamedSharding(mesh, PartitionSpec("core"))
    zeros = [
        np.zeros((n_cores * a.shape[0], *a.shape[1:]), a.dtype) for a in out_avals
    ]
    return fn, sharding, in_names, out_avals, zeros


_CACHE = {}


def _get_runner():
    if "runner" not in _CACHE:
        nc = build_nc()
        _CACHE["runner"] = _build_runner(nc, N_CORES)
    return _CACHE["runner"]


def kernel(x: np.ndarray, mask: np.ndarray, **_) -> np.ndarray:
    import jax

    x = np.ascontiguousarray(np.asarray(x), dtype=np.float32)
    mask = np.ascontiguousarray(np.asarray(mask))
    assert x.shape == (B, C, FM, H, W), x.shape
    assert mask.shape == (B, C, FM, H, W), mask.shape

    fn, sharding, in_names, out_avals, zeros = _get_runner()
    # Batch-sharding == row-blocks of the flat [N_CORES*NT, P, F] view, so
    # the global concatenated operand is just a zero-copy reshape of the
    # full input.  {0,1} bool bytes -> {0,2} u8 folds the dropout scale
    # into the mask (one cheap byte-op pass).
    global_in = {
        "x": x.reshape(N_CORES * NT, P, F),
        "mask": (mask.view(np.uint8) << 1).reshape(N_CORES * NT, P, F),
    }
    if "zeros_dev" not in _CACHE:
        # Output buffers are fully overwritten by the kernel; stage the
        # operand once and reuse it across calls (not donated).
        _CACHE["zeros_dev"] = [jax.device_put(z, sharding) for z in zeros]
    args = [jax.device_put(global_in[n], sharding) for n in in_names]
    args += _CACHE["zeros_dev"]
    out = jax.block_until_ready(fn(*args))
    return np.asarray(out[0]).reshape(B, C, FM, H, W)


# revision 13
# speedup vs baseline: 36.3598x; 1.0107x over previous
"""Inverted-dropout kernel for Trainium2, distributed over 8 NeuronCores.

Computes out = where(mask, x * 2.0, 0) for x:(64,2048,4,7,7) f32 and
mask:(64,2048,4,7,7) bool.  Pure elementwise: shard along batch (8 per core).
Each core streams its shard HBM->SBUF in [128, 3584] f32 tiles, applies one
fused DVE TensorTensor op (the host pre-folds the 1/(1-p)=2.0 dropout scale
into the byte mask, so the op is just x * mask2 with mask2 in {0,2} read
directly as uint8), and streams the result back.  Loads/stores alternate
between the two HWDGE rings (SP / ACT) to balance DMA bytes across rings.

Measured ~68-70 us per core steady-state (~28.9 MB HBM traffic per core at
~420 GB/s — at the practical HBM/fabric ceiling; TimelineSim predicts 84 us).
"""

import sys

import numpy as np

try:
    import concourse.bacc as bacc
except ImportError:  # grading env without the default sys.path site config
    for p in ("/root/.axon_site/_ro/trn_rl_repo", "/opt/trn_rl_repo"):
        if p not in sys.path:
            sys.path.append(p)
    import concourse.bacc as bacc

import concourse.mybir as mybir
from concourse.tile import TileContext

# Full problem shape (hardcoded per harness contract).
B, C, FM, H, W = 64, 2048, 4, 7, 7
N_CORES = 8
B_PER_CORE = B // N_CORES                       # 8
ELEMS_PER_CORE = B_PER_CORE * C * FM * H * W    # 3,211,264 = 7 * 128 * 3584

P = 128          # SBUF partitions
F = 3584         # free-dim elems per tile  (128*3584*4B = 1.75 MiB per x DMA)
NT = ELEMS_PER_CORE // (P * F)                  # 7 tiles
assert NT * P * F == ELEMS_PER_CORE

SCALE = 2.0      # 1 / (1 - p_drop), p_drop = 0.5
BUFS = 6


def build_nc(nt=NT, f=F, bufs=BUFS, repeat=1):
    """Build the per-core SPMD module.

    - Bacc (not bare Bass): Bacc.compile() legalizes sync waits down to the
      TRN2 1-wait-per-instruction limit — walrus rejects the module otherwise.
    - Loads and stores alternate between the two HWDGE rings (SP via nc.sync,
      ACT via nc.scalar) per tile, balancing bytes across both rings
      (~14.5 MB each instead of 16/12.8) — measured ~5% faster than putting
      all loads on one ring.
    - repeat>1 unrolls the whole body R times inside one NEFF (idempotent
      rewrites of the same output), used only for launch-overhead-free timing
      via (T(R2)-T(R1))/(R2-R1).
    """
    nc = bacc.Bacc()
    x = nc.declare_dram_parameter("x", [nt, P, f], mybir.dt.float32, isOutput=False)
    m = nc.declare_dram_parameter("mask", [nt, P, f], mybir.dt.uint8, isOutput=False)
    o = nc.declare_dram_parameter("out", [nt, P, f], mybir.dt.float32, isOutput=True)
    with TileContext(nc) as tc:
        with tc.tile_pool(name="sbuf", bufs=bufs) as pool:
            for _ in range(repeat):
                for i in range(nt):
                    load_eng = nc.sync if i % 2 == 0 else nc.scalar
                    store_eng = nc.scalar if i % 2 == 0 else nc.sync
                    xt = pool.tile([P, f], mybir.dt.float32)
                    load_eng.dma_start(out=xt[:], in_=x[i])
                    mt = pool.tile([P, f], mybir.dt.uint8)
                    load_eng.dma_start(out=mt[:], in_=m[i])
                    ot = pool.tile([P, f], mybir.dt.float32)
                    # mask arrives pre-scaled to {0, 2} (host folds the
                    # 1/(1-p) dropout scale into the byte mask), so one
                    # TensorTensor op computes the whole dropout.
                    nc.vector.tensor_mul(out=ot[:], in0=xt[:], in1=mt[:])
                    store_eng.dma_start(out=o[i], in_=ot[:])
    nc.compile()
    return nc


def _build_runner(nc, n_cores):
    """Compile the SPMD module into a reusable shard_map-jitted callable.

    Same machinery as bass2jax.run_bass_via_pjrt, but the jitted function is
    built once and cached so repeated kernel() calls skip XLA re-tracing.
    Output-buffer donation is dropped: this kernel writes every output
    element, so zero-initialized outputs are unnecessary.
    """
    import jax
    from jax.sharding import Mesh, PartitionSpec, NamedSharding
    from jax.experimental.shard_map import shard_map
    from concourse.bass2jax import (
        _bass_exec_p,
        install_neuronx_cc_hook,
        partition_id_tensor,
    )

    install_neuronx_cc_hook()
    partition_name = nc.partition_id_tensor.name if nc.partition_id_tensor else None

    in_names, out_names, out_avals = [], [], []
    for alloc in nc.m.functions[0].allocations:
        if not isinstance(alloc, mybir.MemoryLocationSet):
            continue
        name = alloc.memorylocations[0].name
        if alloc.kind == "ExternalInput":
            if name != partition_name:
                in_names.append(name)
        elif alloc.kind == "ExternalOutput":
            out_names.append(name)
            out_avals.append(
                jax.core.ShapedArray(
                    tuple(alloc.tensor_shape), mybir.dt.np(alloc.dtype)
                )
            )
    n_params = len(in_names)
    all_in_names = list(in_names) + list(out_names)
    if partition_name is not None:
        all_in_names.append(partition_name)

    def _body(*args):
        operands = list(args)
        if partition_name is not None:
            operands.append(partition_id_tensor())
        outs = _bass_exec_p.bind(
            *operands,
            out_avals=tuple(out_avals),
            in_names=tuple(all_in_names),
            out_names=tuple(out_names),
            lowering_input_output_aliases=(),
            sim_require_finite=True,
            sim_require_nnan=True,
            nc=nc,
        )
        return tuple(outs)

    devices = jax.devices()[:n_cores]
    assert len(devices) == n_cores, (
        f"need {n_cores} devices, have {len(jax.devices())}"
    )
    mesh = Mesh(np.asarray(devices), ("core",))
    in_specs = (PartitionSpec("core"),) * (n_params + len(out_names))
    out_specs = (PartitionSpec("core"),) * len(out_names)
    fn = jax.jit(
        shard_map(
            _body, mesh=mesh, in_specs=in_specs, out_specs=out_specs,
            check_rep=False,
        ),
        keep_unused=True,
    )
    sharding = NamedSharding(mesh, PartitionSpec("core"))
    zeros = [
        np.zeros((n_cores * a.shape[0], *a.shape[1:]), a.dtype) for a in out_avals
    ]
    return fn, sharding, in_names, out_avals, zeros


_CACHE = {}


def _get_runner():
    if "runner" not in _CACHE:
        nc = build_nc()
        _CACHE["runner"] = _build_runner(nc, N_CORES)
    return _CACHE["runner"]


def kernel(x: np.ndarray, mask: np.ndarray, **_) -> np.ndarray:
    import jax

    x = np.ascontiguousarray(np.asarray(x), dtype=np.float32)
    mask = np.asarray(mask)
    if mask.dtype.itemsize != 1:
        mask = mask.astype(np.bool_)
    mask = np.ascontiguousarray(mask)
    assert x.shape == (B, C, FM, H, W), x.shape
    assert mask.shape == (B, C, FM, H, W), mask.shape

    fn, sharding, in_names, out_avals, zeros = _get_runner()
    # Batch-sharding == row-blocks of the flat [N_CORES*NT, P, F] view, so
    # the global concatenated operand is just a zero-copy reshape of the
    # full input.  {0,1} bool bytes -> {0,2} u8 folds the dropout scale
    # into the mask (one cheap byte-op pass).
    global_in = {
        "x": x.reshape(N_CORES * NT, P, F),
        "mask": (mask.view(np.uint8) << 1).reshape(N_CORES * NT, P, F),
    }
    if "zeros_dev" not in _CACHE:
        # Output buffers are fully overwritten by the kernel; stage the
        # operand once and reuse it across calls (not donated).
        _CACHE["zeros_dev"] = [jax.device_put(z, sharding) for z in zeros]
    args = [jax.device_put(global_in[n], sharding) for n in in_names]
    args += _CACHE["zeros_dev"]
    out = jax.block_until_ready(fn(*args))
    return np.asarray(out[0]).reshape(B, C, FM, H, W)
